# revision 21
# baseline (speedup 1.0000x reference)
"""CrossAttention Trainium2 kernel (8 NeuronCores, SPMD), bf16 compute.

Sharding: data-parallel over batch B=2, tensor-parallel over the 16 heads in
4 groups of 4 heads -> 8 cores, one (batch, head-group) pair each. Each core
computes its 4 heads' Q/K/V projections, masked softmax cross-attention, and
its partial output projection y_g = softmax(q k^T * scale) v @ Wo[:, g].T.
The host sums the 4 partial outputs per batch (the Wo row-split all-reduce,
done at unshard time) and adds the v-bias term Wo @ b_v, which is constant
across rows and factors out of the attention (softmax rows sum to 1).

Numerics: inputs are cast to bf16 on the host; every matmul runs bf16 x bf16
with fp32 PSUM accumulation; softmax statistics (denominator, reciprocal,
normalization) stay fp32. End-to-end relative error ~3e-3.

Layout: the PE contracts over the partition dim, so activations and weights
are laid out contraction-major. The host passes x/context/weight shards
already transposed (contraction axis leading) so every device DMA is a plain
contiguous row load; there are no transposes anywhere on the device.

Attention is computed scores-transposed: ST[m, n] per head, so the PV matmul
contracts over m directly. The softmax denominator comes for free from an
appended ones-column on the v stationary operand. exp() is unnormalized (no
max subtraction; scores*scale are bounded, |s| < ~4); mask zeros are applied
multiplicatively after exp.

Phase-B staging keeps the PE dense (HAM-warm) and overlaps the ACT-bound
exp stream with PE work:
  stage 1: scores+exp+mask for heads 0,1 (ACT-bound) with the V projection
           interleaved on the otherwise idle PE; masked exps parked in SBUF.
  stage 2: PV accumulation for heads 0,1 (dense PE) interleaved with
           scores+exp+mask for heads 2,3.
  stage 3: PV accumulation for heads 2,3, overlapped with the softmax
           normalization of heads 0,1.
"""

import os

import numpy as np
import ml_dtypes

import concourse.bass as bass
import concourse.bacc as bacc
import concourse.mybir as mybir
import concourse.tile as tile
from concourse.bass_utils import run_bass_kernel_spmd

DIM = 1024
HEAD_DIM = 64
NUM_HEADS = 16
SCALE = HEAD_DIM**-0.5
B, N, M = 2, 1024, 2048
HPC = 4  # heads per core
E = HPC * HEAD_DIM  # 256: per-core projection width
P = 128
F32 = mybir.dt.float32
BF16 = mybir.dt.bfloat16
CT = DIM // P  # 8 contraction tiles
MT = M // P  # 16 m tiles


def _bc_heads(ap):
    """Broadcast a [P, N] AP to [P, 2, N] with a zero-stride head dim."""
    return bass.AP(ap.tensor, ap.offset, [ap.ap[0], [0, 2], ap.ap[1]])


def build_program():
    nc = bacc.Bacc("TRN2", target_bir_lowering=False, debug=False, num_devices=8)

    # all activation/weight shards arrive contraction-major (pre-transposed)
    xT_d = nc.dram_tensor("xT", [DIM, N], BF16, kind="ExternalInput").ap()
    ctxT_d = nc.dram_tensor("ctxT", [DIM, M], BF16, kind="ExternalInput").ap()
    maskt_d = nc.dram_tensor("maskt", [M, N], BF16, kind="ExternalInput").ap()
    wqT_d = nc.dram_tensor("wqT", [DIM, E], BF16, kind="ExternalInput").ap()
    wkT_d = nc.dram_tensor("wkT", [DIM, E], BF16, kind="ExternalInput").ap()
    wvT_d = nc.dram_tensor("wvT", [DIM, E], BF16, kind="ExternalInput").ap()
    woT_d = nc.dram_tensor("woT", [E, DIM], BF16, kind="ExternalInput").ap()
    bk_d = nc.dram_tensor("bk", [E], F32, kind="ExternalInput").ap()
    y_d = nc.dram_tensor("y", [N, DIM], F32, kind="ExternalOutput").ap()

    kdbg = bool(os.environ.get("KDBG"))
    if kdbg:
        otdump_d = nc.dram_tensor(
            "otdump", [HEAD_DIM + 1, HPC, N], F32, kind="ExternalOutput"
        ).ap()

    Exp = mybir.ActivationFunctionType.Exp

    from contextlib import ExitStack

    with tile.TileContext(nc) as tc, ExitStack() as ctx:
        const = ctx.enter_context(tc.tile_pool(name="const", bufs=1))
        bk_sb = const.tile([P, E // P], F32)
        nc.sync.dma_start(out=bk_sb, in_=bk_d.rearrange("(t p) -> p t", p=P))

        persist = ctx.enter_context(tc.tile_pool(name="persist", bufs=1))
        qT = persist.tile([P, E // P, N], BF16)
        kT = persist.tile([P, E // P, M], BF16)
        vaug = persist.tile([P, MT, HPC, HEAD_DIM + 1], BF16)
        woT = persist.tile([P, E // P, DIM], BF16)
        # rows 0:64 unnormalized attention out, row 64 denominator
        ot_sb = persist.tile([HEAD_DIM + 1, HPC, N], F32)
        otn2 = persist.tile([P, E // P, N], BF16)

        # ones column: fill everything; v evictions overwrite cols 0:64
        nc.vector.memset(vaug, 1.0)

        bwork = ctx.enter_context(tc.tile_pool(name="bwork", bufs=4))
        maskp = ctx.enter_context(tc.tile_pool(name="maskp", bufs=3))
        rbp = ctx.enter_context(tc.tile_pool(name="rbp", bufs=2))

        def emit_scores(spool, sbufs, hp, mt, exmst, mk):
            """scores -> exp -> mask for head pair hp at m-tile mt,
            per n-chunk PSUM tiles so the next tile can double-buffer."""
            for chn in range(N // 512):
                st = spool.tile(
                    [P, 2, 512], F32, tag="st", name="st", bufs=sbufs
                )
                for hl in range(2):
                    erow = slice(hl * HEAD_DIM, (hl + 1) * HEAD_DIM)
                    nc.tensor.matmul(
                        st[:, hl, :],
                        lhsT=kT[erow, hp, mt * P : (mt + 1) * P],
                        rhs=qT[erow, hp, chn * 512 : (chn + 1) * 512],
                        start=True,
                        stop=True,
                    )
                ex = bwork.tile([P, 2, 512], BF16, tag="ex", name="ex")
                nc.scalar.activation(ex, st, Exp, scale=float(SCALE))
                mks = mk[:, chn * 512 : (chn + 1) * 512]
                mkc = bass.AP(mks.tensor, mks.offset, [mks.ap[0], [0, 2], mks.ap[1]])
                nc.vector.tensor_mul(
                    exmst[:, mt, :, chn * 512 : (chn + 1) * 512], ex, mkc
                )

        def emit_pv(ot_ps, hp, mt, exmst):
            for hl in range(2):
                h = hp * 2 + hl
                for chn in range(N // 512):
                    nc.tensor.matmul(
                        ot_ps[hl * 2 + chn],
                        lhsT=vaug[:, mt, h, :],
                        rhs=exmst[:, mt, hl, chn * 512 : (chn + 1) * 512],
                        start=(mt == 0),
                        stop=(mt == MT - 1),
                    )

        def evict_ot(ot_ps, hp):
            for hl in range(2):
                for chn in range(2):
                    nc.vector.tensor_copy(
                        ot_sb[:, hp * 2 + hl, chn * 512 : (chn + 1) * 512],
                        ot_ps[hl * 2 + chn],
                    )

        def normalize_head(h, dn_pool):
            """softmax-normalize head h into its otn2 half."""
            hp, hl = divmod(h, 2)
            dn = slice(HEAD_DIM, HEAD_DIM + 1)
            # partition_broadcast only reads partition 0 on HW: move the
            # denominator row (partition 64) to partition 0 via SBUF DMA.
            dn_sb = dn_pool.tile([1, N], F32, tag="dn", name="dn", bufs=2)
            nc.sync.dma_start(out=dn_sb, in_=ot_sb[dn, h, :])
            rbr = rbp.tile([HEAD_DIM, N], F32, tag="rbr", name="rbr")
            nc.gpsimd.partition_broadcast(rbr, dn_sb[0:1, :])
            rb = rbp.tile([HEAD_DIM, N], F32, tag="rb", name="rb")
            nc.vector.reciprocal_approx_fast(out=rb, in_=rbr)
            if hl == 0:
                nc.vector.tensor_mul(
                    otn2[:HEAD_DIM, hp, :], ot_sb[:HEAD_DIM, h, :], rb
                )
            else:
                tmp = rbp.tile([HEAD_DIM, N], BF16, tag="tmp", name="tmp")
                nc.vector.tensor_mul(tmp, ot_sb[:HEAD_DIM, h, :], rb)
                # partition shift 0:64 -> 64:128 via SBUF-SBUF DMA
                nc.sync.dma_start(out=otn2[HEAD_DIM:P, hp, :], in_=tmp)

        def load_mask(mt):
            mk = maskp.tile([P, N], BF16, tag="mk", name="mk")
            nc.gpsimd.dma_start(out=mk, in_=maskt_d[mt * P : (mt + 1) * P, :])
            return mk

        with tc.tile_pool(name="exmp", bufs=1) as exmp:
            # masked exp(scores) parked per m-tile; one buffer reused across
            # head pairs (WAR: stage-2 rewrites a tile only after its PV read)
            exmst = exmp.tile([P, MT, 2, N], BF16)

            with tc.tile_pool(name="wctx", bufs=1) as wctx_pool:
                wkT = wctx_pool.tile([P, CT, E], BF16)
                wvT = wctx_pool.tile([P, CT, E], BF16)
                ctxT = wctx_pool.tile([P, CT, M], BF16)

                with tc.tile_pool(name="qx", bufs=1) as qx_pool:
                    wqT = qx_pool.tile([P, CT, E], BF16)
                    xT = qx_pool.tile([P, CT, N], BF16)
                    # batched contiguous loads (one DMA per tensor),
                    # dependency-first; ctx-side tensors ride the second
                    # HWDGE ring (scalar) in parallel
                    nc.sync.dma_start(
                        out=wqT, in_=wqT_d.rearrange("(j p) e -> p j e", p=P)
                    )
                    nc.sync.dma_start(
                        out=xT, in_=xT_d.rearrange("(j p) n -> p j n", p=P)
                    )
                    nc.scalar.dma_start(
                        out=wkT, in_=wkT_d.rearrange("(j p) e -> p j e", p=P)
                    )
                    nc.scalar.dma_start(
                        out=ctxT, in_=ctxT_d.rearrange("(j p) m -> p j m", p=P)
                    )
                    nc.scalar.dma_start(
                        out=wvT, in_=wvT_d.rearrange("(j p) e -> p j e", p=P)
                    )
                    nc.scalar.dma_start(
                        out=woT, in_=woT_d.rearrange("(t p) o -> p t o", p=P)
                    )

                    # Q projection
                    with tc.tile_pool(name="ppsA", bufs=3, space="PSUM") as ppsA:
                        for et in range(E // P):
                            for chn in range(N // 512):
                                pq = ppsA.tile([P, 512], F32, tag="pq")
                                for j in range(CT):
                                    nc.tensor.matmul(
                                        pq,
                                        lhsT=wqT[:, j, et * P : (et + 1) * P],
                                        rhs=xT[:, j, chn * 512 : (chn + 1) * 512],
                                        start=(j == 0),
                                        stop=(j == CT - 1),
                                    )
                                nc.vector.tensor_copy(
                                    qT[:, et, chn * 512 : (chn + 1) * 512], pq
                                )

                def emit_kproj(kps, et, chm):
                    pk = kps.tile([P, 512], F32, tag="pk", name="pk")
                    for j in range(CT):
                        nc.tensor.matmul(
                            pk,
                            lhsT=wkT[:, j, et * P : (et + 1) * P],
                            rhs=ctxT[:, j, chm * 512 : (chm + 1) * 512],
                            start=(j == 0),
                            stop=(j == CT - 1),
                        )
                    nc.vector.tensor_scalar_add(
                        kT[:, et, chm * 512 : (chm + 1) * 512],
                        pk,
                        bk_sb[:, et : et + 1],
                    )

                # K projection for the first head pair's first chunk must
                # precede stage 1; the rest is folded into stage 1's PE slack.
                # stage 1: scores(heads 0,1) [ACT-bound] + V and K
                # projections interleaved on the otherwise idle PE.
                with (
                    tc.tile_pool(name="sps1", bufs=1, space="PSUM") as sps1,
                    tc.tile_pool(name="vps", bufs=2, space="PSUM") as vps,
                    tc.tile_pool(name="kps", bufs=2, space="PSUM") as kps,
                ):
                    emit_kproj(kps, 0, 0)
                    for mt in range(MT):
                        # keep kT(et0) one chunk ahead of the scores that
                        # consume it; kT(et1) lands before stage 2
                        if mt % 2 == 0:
                            et, chm = divmod(mt // 2 + 1, M // 512)
                            if et < 2:
                                emit_kproj(kps, et, chm)
                        mk = load_mask(mt)
                        emit_scores(sps1, 2, 0, mt, exmst, mk)
                        pv = vps.tile([P, E], F32, tag="pv")
                        for j in range(CT):
                            nc.tensor.matmul(
                                pv,
                                lhsT=ctxT[:, j, mt * P : (mt + 1) * P],
                                rhs=wvT[:, j, :],
                                start=(j == 0),
                                stop=(j == CT - 1),
                            )
                        for h in range(HPC):
                            nc.vector.tensor_copy(
                                vaug[:, mt, h, :HEAD_DIM],
                                pv[:, h * HEAD_DIM : (h + 1) * HEAD_DIM],
                            )

            # stage 2: PV(heads 0,1) interleaved with scores(heads 2,3)
            with tc.tile_pool(name="ops0", bufs=1, space="PSUM") as ops0:
                ot_ps0 = [
                    ops0.tile([HEAD_DIM + 1, 512], F32, tag=f"o{i}", name=f"o{i}")
                    for i in range(4)
                ]
                with tc.tile_pool(name="sps2", bufs=1, space="PSUM") as sps2:
                    for mt in range(MT):
                        mk = load_mask(mt)
                        emit_pv(ot_ps0, 0, mt, exmst)
                        emit_scores(sps2, 2, 1, mt, exmst, mk)
                evict_ot(ot_ps0, 0)

            # stage 3: PV per head (2 then 3); normalization of earlier heads
            # overlaps the remaining PV sweeps
            with (
                tc.tile_pool(name="ops1", bufs=1, space="PSUM") as ops1,
                tc.tile_pool(name="dnp", bufs=1) as dnp,
            ):
                ot_ps1 = [
                    ops1.tile([HEAD_DIM + 1, 512], F32, tag=f"p{i}", name=f"p{i}")
                    for i in range(4)
                ]
                normalize_head(0, dnp)
                normalize_head(1, dnp)
                for hl in range(2):
                    h = 2 + hl
                    for mt in range(MT):
                        for chn in range(N // 512):
                            nc.tensor.matmul(
                                ot_ps1[hl * 2 + chn],
                                lhsT=vaug[:, mt, h, :],
                                rhs=exmst[
                                    :, mt, hl, chn * 512 : (chn + 1) * 512
                                ],
                                start=(mt == 0),
                                stop=(mt == MT - 1),
                            )
                    for chn in range(2):
                        nc.vector.tensor_copy(
                            ot_sb[:, h, chn * 512 : (chn + 1) * 512],
                            ot_ps1[hl * 2 + chn],
                        )
                    normalize_head(h, dnp)

            if kdbg:
                nc.sync.dma_start(out=otdump_d, in_=ot_sb)

        # ---------- output projection ----------
        with (
            tc.tile_pool(name="ypsum", bufs=3, space="PSUM") as ypsum,
            tc.tile_pool(name="ypool", bufs=3) as ypool,
        ):
            for nb in range(N // P):
                for oc in range(DIM // 512):
                    yp = ypsum.tile([P, 512], F32, tag="yp")
                    for hp in range(E // P):
                        nc.tensor.matmul(
                            yp,
                            lhsT=otn2[:, hp, nb * P : (nb + 1) * P],
                            rhs=woT[:, hp, oc * 512 : (oc + 1) * 512],
                            start=(hp == 0),
                            stop=(hp == E // P - 1),
                        )
                    ys = ypool.tile([P, 512], F32, tag="ys")
                    nc.vector.tensor_copy(ys, yp)
                    ring = nc.scalar if (nb + oc) % 2 else nc.sync
                    ring.dma_start(
                        out=y_d[nb * P : (nb + 1) * P, oc * 512 : (oc + 1) * 512],
                        in_=ys,
                    )

    nc.compile()
    return nc


_NC_CACHE = []


def _get_nc():
    if not _NC_CACHE:
        _NC_CACHE.append(build_program())
    return _NC_CACHE[0]


def make_in_maps(x, context, mask, Wq, Wkv, b_kv, Wo):
    bf = ml_dtypes.bfloat16
    x = np.asarray(x, dtype=np.float32)
    context = np.asarray(context, dtype=np.float32)
    mask = np.asarray(mask)
    Wq = np.asarray(Wq, dtype=np.float32)
    Wkv = np.asarray(Wkv, dtype=np.float32)
    b_kv = np.asarray(b_kv, dtype=np.float32)
    Wo = np.asarray(Wo, dtype=np.float32)

    in_maps = []
    for b in range(B):
        xtb = np.ascontiguousarray(x[b].T).astype(bf)
        ctb = np.ascontiguousarray(context[b].T).astype(bf)
        mtb = np.ascontiguousarray(mask[b].T).astype(bf)
        for g in range(NUM_HEADS // HPC):
            sl = slice(E * g, E * (g + 1))
            in_maps.append(
                {
                    "xT": xtb,
                    "ctxT": ctb,
                    "maskt": mtb,
                    "wqT": np.ascontiguousarray(Wq[sl].T).astype(bf),
                    "wkT": np.ascontiguousarray(Wkv[sl].T).astype(bf),
                    "wvT": np.ascontiguousarray(
                        Wkv[DIM + E * g : DIM + E * (g + 1)].T
                    ).astype(bf),
                    "woT": np.ascontiguousarray(Wo[:, sl].T).astype(bf),
                    "bk": np.ascontiguousarray(b_kv[sl]),
                }
            )
    return in_maps


def combine_outputs(ys, b_kv, Wo):
    """ys: list of 8 per-core partial outputs [N, DIM], core order (b, g)."""
    b_v = np.asarray(b_kv, dtype=np.float32)[DIM:]
    ybias = np.asarray(Wo, dtype=np.float32) @ b_v  # [DIM]
    out = np.empty((B, N, DIM), dtype=np.float32)
    G = NUM_HEADS // HPC
    for b in range(B):
        acc = np.asarray(ys[G * b], dtype=np.float32).copy()
        for g in range(1, G):
            acc += ys[G * b + g]
        out[b] = acc + ybias[None, :]
    return out


def kernel(x, context, mask, Wq, Wkv, b_kv, Wo):
    nc = _get_nc()
    in_maps = make_in_maps(x, context, mask, Wq, Wkv, b_kv, Wo)
    res = run_bass_kernel_spmd(nc, in_maps, core_ids=list(range(8)))
    ys = [m["y"] for m in res.results]
    return combine_outputs(ys, b_kv, Wo)
